# revision 1
# baseline (speedup 1.0000x reference)
"""GAT layer (PyG-style, add_self_loops=True) on 8 Trainium2 NeuronCores.

Strategy (per sharding hint): partition destination nodes (and their incident
edges) across the 8 cores; each core owns a contiguous range of 6250 dst nodes.

Per core:
  phase 1: full projection table row[n] = [h(256) | a_src(4) | pad] (f32,
           1280 B rows) in local DRAM -- replicated compute, zero cross-core
           communication.  Split into TWO tables (lo: nodes < 25088, hi: rest)
           because dma_gather indices are int16 and its in_ap base offset is
           ignored by the Q7 ucode.
  phase 1b: a_dst for the core's own 6272 dst nodes from a host-sliced
           x[dst_shard].T input -- lives entirely in SBUF ([128, 49*4]).
  phase 2: per window of 128 dst nodes, two dma_gathers (lo/hi edge lists,
           padded to a compile-time-uniform subtile count) pull the source
           rows for all incident edges.  exp(leaky_relu(a_src[src]+a_dst[dst]))
           is built with vector ops; a one-hot (edge -> dst slot) matrix turns
           the segment-sum of [e*h | e] into PSUM-accumulated matmuls; final
           out = acc/(denom+eps) + bias.  Softmax max-subtraction is skipped
           (shift-invariant; scores are O(1) so exp cannot overflow).

Pad edges point at a dummy table row with a_src = -1e30 => exp score exactly 0.

Host does only index-space work (self-loop append, dst sort, windowing,
padding, int16 index wrapping) plus data layout (x transposed/sliced).

HW constraint baked in throughout: one semaphore wait per instruction (bacc
generate_event_semaphores splits most, but PE matmuls keep a single wait), so
every tile a matmul reads is produced on DVE, and PSUM slots are read on DVE.
"""

import math

import numpy as np

N = 50000
IN_DIM = 64
H = 4
D = 64
HD = H * D  # 256
ROWC = 384  # bf16 table row: h(256) | a_src hi(4) | a_src lo(4) | pad = 768 B
WCOLS = HD + 2 * H  # 264: phase-1 matmul output h | a_src | a_dst
NEG_SLOPE = 0.2
EPS = 1e-16

NCORES = 8
NPC = N // NCORES  # 6250 dst nodes per core
NWIN = math.ceil(NPC / 128)  # 49 windows
WROWS = NWIN * 128  # 6272
NT1 = 392  # phase-1 tiles (50176 nodes incl. pad)
NROWS_ALL = NT1 * 128  # 50176
SPLIT_T = 196  # lo/hi table split, in 128-row tiles
SPLIT = SPLIT_T * 128  # 25088
LO_TILES = SPLIT_T + 1  # +1 dummy tile
LO_ROWS = LO_TILES * 128  # 25216
HI_TILES = NT1 - SPLIT_T  # 196
HI_ROWS = HI_TILES * 128  # 25088
DUMMY_LO = SPLIT  # row 25088 of lo table (dedicated dummy row)
DUMMY_HI = N - SPLIT  # row 24912 of hi table (= node 50000, h == 0)
SUBS = 4  # edge subtiles per chunk
B1 = 7  # phase-1 tiles per iteration (divides both 196 and 392)

LAST_RESULTS = None  # BassKernelResults of the most recent run (for test.py)


def _wrap_idx(ids):
    """[n] int -> dma_gather wrapped layout [128, n/16] int16
    (idx i at [i%16, i//16], replicated across the 8 Q7 core groups)."""
    n = len(ids)
    w16 = ids.reshape(n // 16, 16).T.astype(np.int16)  # [16, n/16]
    return np.tile(w16, (8, 1))


def _prep_host(edge_index):
    """Returns ilow  int16 [NCORES, NWIN, 128, KL*8]
               ihigh int16 [NCORES, NWIN, 128, KH*8]
               dstrel f32  [NCORES, NWIN, 128, KL+KH]
               (KL, KH)"""
    src = np.concatenate([edge_index[0], np.arange(N, dtype=np.int64)]).astype(np.int64)
    dst = np.concatenate([edge_index[1], np.arange(N, dtype=np.int64)]).astype(np.int64)
    order = np.argsort(dst, kind="stable")
    src = src[order].astype(np.int32)
    dst = dst[order].astype(np.int32)

    bounds = [c * NPC + w * 128 for c in range(NCORES) for w in range(NWIN)]
    bounds.append(N)
    cuts = np.searchsorted(dst, np.asarray(bounds))

    # per-(core,window) lo/hi counts -> uniform padded subtile counts
    lo_counts = np.zeros(NCORES * NWIN, np.int64)
    hi_counts = np.zeros(NCORES * NWIN, np.int64)
    for b in range(NCORES * NWIN):
        s = src[cuts[b] : cuts[b + 1]]
        lo_counts[b] = int((s < SPLIT).sum())
        hi_counts[b] = len(s) - lo_counts[b]
    KL = max(1, math.ceil(lo_counts.max() / 128))
    KH = max(1, math.ceil(hi_counts.max() / 128))
    kj = KL + KH

    ilow = np.full((NCORES, NWIN, KL * 128), DUMMY_LO, np.int32)
    ihigh = np.full((NCORES, NWIN, KH * 128), DUMMY_HI, np.int32)
    dstrel = np.zeros((NCORES, NWIN, 128, kj), np.float32)
    for c in range(NCORES):
        base = c * NPC
        for w in range(NWIN):
            b = c * NWIN + w
            s = src[cuts[b] : cuts[b + 1]]
            d = dst[cuts[b] : cuts[b + 1]] - base - w * 128
            m = s < SPLIT
            slo, dlo = s[m], d[m]
            shi, dhi = s[~m] - SPLIT, d[~m]
            # ascending source rows => HBM page locality in the gather
            o = np.argsort(slo, kind="stable")
            slo, dlo = slo[o], dlo[o]
            o = np.argsort(shi, kind="stable")
            shi, dhi = shi[o], dhi[o]
            ilow[c, w, : len(slo)] = slo
            ihigh[c, w, : len(shi)] = shi
            i = np.arange(len(slo))
            dstrel[c, w, i % 128, i // 128] = dlo
            i = np.arange(len(shi))
            dstrel[c, w, i % 128, KL + i // 128] = dhi
    ilow_w = np.zeros((NCORES, NWIN, 128, KL * 8), np.int16)
    ihigh_w = np.zeros((NCORES, NWIN, 128, KH * 8), np.int16)
    for c in range(NCORES):
        for w in range(NWIN):
            ilow_w[c, w] = _wrap_idx(ilow[c, w])
            ihigh_w[c, w] = _wrap_idx(ihigh[c, w])
    return ilow_w, ihigh_w, dstrel, KL, KH


def _build_program(KL, KH, ablate="full"):
    import concourse.bass as bass
    import concourse.bacc as bacc
    import concourse.tile as tile
    from concourse import mybir
    from concourse.masks import make_identity

    f32 = mybir.dt.float32
    bf16 = mybir.dt.bfloat16
    i16 = mybir.dt.int16
    i32 = mybir.dt.int32
    kj = KL + KH

    nc = bacc.Bacc(None, target_bir_lowering=False)

    xT_d = nc.dram_tensor("xT", [IN_DIM, NROWS_ALL], f32, kind="ExternalInput")
    xdT_d = nc.dram_tensor("xdstT", [IN_DIM, WROWS], f32, kind="ExternalInput")
    W_d = nc.dram_tensor("W", [IN_DIM, HD], f32, kind="ExternalInput")
    asrc_d = nc.dram_tensor("att_src", [1, HD], f32, kind="ExternalInput")
    adst_d = nc.dram_tensor("att_dst", [1, HD], f32, kind="ExternalInput")
    bias_d = nc.dram_tensor("bias", [1, HD], f32, kind="ExternalInput")
    il_d = nc.dram_tensor("ilow", [NWIN, 128, KL * 8], i16, kind="ExternalInput")
    ih_d = nc.dram_tensor("ihigh", [NWIN, 128, KH * 8], i16, kind="ExternalInput")
    drel_d = nc.dram_tensor("dstrel", [NWIN, 128, kj], f32, kind="ExternalInput")
    out_d = nc.dram_tensor("out", [WROWS, HD], f32, kind="ExternalOutput")
    tbl_lo = nc.dram_tensor("tbl_lo", [LO_ROWS, ROWC], bf16)  # 640 B rows
    tbl_hi = nc.dram_tensor("tbl_hi", [HI_ROWS, ROWC], bf16)

    X = mybir.AxisListType.X
    EQ = mybir.AluOpType.is_equal
    MULT = mybir.AluOpType.mult
    MAX = mybir.AluOpType.max

    with tile.TileContext(nc) as tc:
        with tc.tile_pool(name="const", bufs=1) as cpool:
            spsum_cm = tc.tile_pool(name="setup_psum", bufs=1, space="PSUM")
            spsum = spsum_cm.__enter__()
            ones = cpool.tile([1, 128], f32)
            nc.vector.memset(ones[:], 1.0)
            ident_src = cpool.tile([128, 128], f32)
            make_identity(nc, ident_src[:])
            ident = cpool.tile([128, 128], f32)
            nc.vector.tensor_copy(ident[:], ident_src[:])
            iota_i = cpool.tile([128, 128], i32)
            nc.gpsimd.iota(iota_i[:], pattern=[[1, 128]], base=0, channel_multiplier=0)
            iota_f = cpool.tile([128, 128], f32)
            nc.vector.tensor_copy(iota_f[:], iota_i[:])

            # WA = [W | Wsrc | Wdst | 0...], Wsrc[k,h] = sum_d W[k,h*D+d]*att_src[h,d]
            # padded to ROWC cols so phase-1 writes full table rows.
            wa_tmp = cpool.tile([IN_DIM, ROWC], f32)
            nc.vector.memset(wa_tmp[:], 0.0)
            nc.sync.dma_start(wa_tmp[:, 0:HD], W_d[:, :])
            att_s_raw = cpool.tile([1, HD], f32)
            nc.sync.dma_start(att_s_raw[:], asrc_d[:, :])
            att_t_raw = cpool.tile([1, HD], f32)
            nc.sync.dma_start(att_t_raw[:], adst_d[:, :])
            att_s = cpool.tile([1, HD], f32)
            nc.vector.tensor_copy(att_s[:], att_s_raw[:])
            att_t = cpool.tile([1, HD], f32)
            nc.vector.tensor_copy(att_t[:], att_t_raw[:])
            for att_tile, col0 in ((att_s, HD), (att_t, HD + H)):
                attb = spsum.tile([IN_DIM, HD], f32, tag="attb")
                nc.tensor.matmul(
                    attb[:], lhsT=ones[:1, 0:IN_DIM], rhs=att_tile[:],
                    start=True, stop=True,
                )
                tmp = cpool.tile([IN_DIM, HD], f32, tag="tmp")
                nc.vector.tensor_mul(tmp[:], wa_tmp[:, 0:HD], attb[:])
                nc.vector.reduce_sum(
                    out=wa_tmp[:, col0 : col0 + H],
                    in_=tmp[:].rearrange("k (h d) -> k h d", d=D),
                    axis=X,
                )
            WA = cpool.tile([IN_DIM, ROWC], f32)
            nc.vector.tensor_copy(WA[:], wa_tmp[:])

            bias_raw = cpool.tile([1, HD], f32)
            nc.sync.dma_start(bias_raw[:], bias_d[:, :])
            bias_sb = cpool.tile([1, HD], f32)
            nc.vector.tensor_copy(bias_sb[:], bias_raw[:])
            bb = spsum.tile([128, HD], f32)
            nc.tensor.matmul(bb[:], lhsT=ones[:1, :], rhs=bias_sb[:], start=True, stop=True)
            bias_bc = cpool.tile([128, HD], f32)
            nc.scalar.copy(bias_bc[:], bb[:])

            # a_dst for the core's own dst shard: [128, NWIN*H] in SBUF
            adst_all = cpool.tile([128, NWIN * H], f32)

            spsum_cm.__exit__(None, None, None)  # free setup PSUM banks

            # ---------------- phase 1: build src tables ----------------
            with (
                tc.tile_pool(name="p1", bufs=4) as p1,
                tc.tile_pool(name="p1ps", bufs=4, space="PSUM") as p1ps,
                tc.tile_pool(name="p1psb", bufs=2, space="PSUM") as p1psb,
            ):
                for it in range(NT1 // B1):
                    t0 = it * B1
                    xt = p1.tile([IN_DIM, B1 * 128], f32, tag="xtr")
                    nc.sync.dma_start(xt[:], xT_d[:, t0 * 128 : (t0 + B1) * 128])
                    hs = p1.tile([128, B1 * ROWC], bf16, tag="hs")
                    for k in range(B1):
                        hp = p1ps.tile([128, ROWC], f32, tag="hp")
                        nc.tensor.matmul(
                            hp[:],
                            lhsT=xt[:, k * 128 : (k + 1) * 128],
                            rhs=WA[:],
                            start=True,
                            stop=True,
                        )
                        # cast h+a_src to bf16 (a_src hi at cols 256:260),
                        # then lo = a_src - hi at cols 260:264 (split-precision)
                        nc.vector.tensor_copy(hs[:, k * ROWC : (k + 1) * ROWC], hp[:])
                        nc.vector.tensor_sub(
                            hs[:, k * ROWC + HD + H : k * ROWC + HD + 2 * H],
                            hp[:, HD : HD + H],
                            hs[:, k * ROWC + HD : k * ROWC + HD + H],
                        )
                    if t0 < SPLIT_T:
                        dst_ap = tbl_lo[t0 * 128 : (t0 + B1) * 128, :]
                    else:
                        u = t0 - SPLIT_T
                        dst_ap = tbl_hi[u * 128 : (u + B1) * 128, :]
                    nc.scalar.dma_start(
                        dst_ap.rearrange("(k p) c -> p k c", p=128),
                        hs[:].rearrange("p (k c) -> p k c", c=ROWC),
                    )

                # phase 1b: a_dst for own dst shard (stays in SBUF)
                for w in range(NWIN):
                    xd = p1.tile([IN_DIM, 128], f32, tag="xdr")
                    nc.sync.dma_start(xd[:], xdT_d[:, w * 128 : (w + 1) * 128])
                    adp = p1psb.tile([128, H], f32, tag="adp")
                    nc.tensor.matmul(
                        adp[:], lhsT=xd[:], rhs=WA[:, HD + H : HD + 2 * H],
                        start=True, stop=True,
                    )
                    nc.vector.tensor_copy(adst_all[:, w * H : (w + 1) * H], adp[:])

            # dummy rows: whole trailing lo tile; a_src of hi dummy (node 50000)
            zrow = cpool.tile([128, ROWC], bf16)
            nc.vector.memset(zrow[:], 0.0)
            nc.vector.memset(zrow[:, HD : HD + H], -1e30)
            nc.sync.dma_start(tbl_lo[DUMMY_LO : DUMMY_LO + 128, :], zrow[:])
            nc.sync.dma_start(
                tbl_hi[DUMMY_HI : DUMMY_HI + 1, HD : HD + 2 * H],
                zrow[:1, HD : HD + 2 * H],
            )

            # ---------------- phase 2: per-window aggregation ----------------
            if ablate == "p1":
                with tc.tile_pool(name="fin0", bufs=1) as f0:
                    zo = f0.tile([128, HD], f32)
                    nc.vector.memset(zo[:], 0.0)
                    for w in range(NWIN):
                        nc.sync.dma_start(out_d[w * 128 : (w + 1) * 128, :], zo[:])
            nch = math.ceil(kj / SUBS)
            with (
                tc.tile_pool(name="gat", bufs=2) as gpool,
                tc.tile_pool(name="edg", bufs=2) as epool,
                tc.tile_pool(name="wrk", bufs=3) as wpool,
                tc.tile_pool(name="fin", bufs=2) as fpool,
                tc.tile_pool(name="p2ps", bufs=2, space="PSUM") as p2ps,
            ):
                for w in range(NWIN if ablate != "p1" else 0):
                    il_t = epool.tile([128, KL * 8], i16, tag="il")
                    nc.sync.dma_start(il_t[:], il_d[w])
                    ih_t = epool.tile([128, KH * 8], i16, tag="ih")
                    nc.sync.dma_start(ih_t[:], ih_d[w])
                    drel_t = epool.tile([128, kj], f32, tag="drel")
                    nc.sync.dma_start(drel_t[:], drel_d[w])

                    g = gpool.tile([128, kj * ROWC], bf16, tag="g")
                    gv = g[:].rearrange("p (j c) -> p j c", c=ROWC)
                    nc.gpsimd.dma_gather(
                        out_ap=gv[:, 0:KL, :], in_ap=tbl_lo[:, :], idxs_ap=il_t[:],
                        num_idxs=KL * 128, num_idxs_reg=KL * 128, elem_size=ROWC,
                        single_packet=False,
                    )
                    nc.gpsimd.dma_gather(
                        out_ap=gv[:, KL:kj, :], in_ap=tbl_hi[:, :], idxs_ap=ih_t[:],
                        num_idxs=KH * 128, num_idxs_reg=KH * 128, elem_size=ROWC,
                        single_packet=False,
                    )
                    adw = adst_all[:, w * H : (w + 1) * H]

                    if ablate == "p1g":
                        outw = fpool.tile([128, HD], f32, tag="outw")
                        nc.vector.tensor_copy(outw[:], g[:, 0:HD])
                        nc.sync.dma_start(out_d[w * 128 : (w + 1) * 128, :], outw[:])
                        continue

                    accdns = p2ps.tile([128, HD + H], f32, tag="accdns")
                    for ch in range(nch):
                        s0 = ch * SUBS
                        ns = min(SUBS, kj - s0)
                        # one-hot[e, s, p] = (dstrel[e, s] == p)
                        oh = wpool.tile([128, SUBS * 128], f32, tag="oh")
                        nc.vector.tensor_tensor(
                            out=oh[:, 0 : ns * 128].rearrange("p (s e) -> p s e", s=ns),
                            in0=drel_t[:, s0 : s0 + ns].unsqueeze(-1).to_broadcast(
                                [128, ns, 128]
                            ),
                            in1=iota_f[:].unsqueeze(1).to_broadcast([128, ns, 128]),
                            op=EQ,
                        )
                        # transposed one-hot (for a_dst expansion)
                        ohT_ps = p2ps.tile([128, SUBS * 128], f32, tag="ohT")
                        for s in range(ns):
                            nc.tensor.transpose(
                                ohT_ps[:, s * 128 : (s + 1) * 128],
                                oh[:, s * 128 : (s + 1) * 128],
                                ident[:],
                            )
                        ohT = wpool.tile([128, SUBS * 128], f32, tag="ohTs")
                        nc.scalar.copy(ohT[:, 0 : ns * 128], ohT_ps[:, 0 : ns * 128])
                        # a_dst per edge
                        adx = p2ps.tile([128, SUBS * H], f32, tag="adx")
                        for s in range(ns):
                            nc.tensor.matmul(
                                adx[:, s * H : (s + 1) * H],
                                lhsT=ohT[:, s * 128 : (s + 1) * 128],
                                rhs=adw,
                                start=True,
                                stop=True,
                            )
                        # scores -> exp(leaky_relu)
                        asr = wpool.tile([128, SUBS * H], f32, tag="asr")
                        nc.vector.tensor_add(
                            asr[:, 0 : ns * H].rearrange("p (s h) -> p s h", h=H),
                            gv[:, s0 : s0 + ns, HD : HD + H],
                            gv[:, s0 : s0 + ns, HD + H : HD + 2 * H],
                        )
                        sc = wpool.tile([128, SUBS * H], f32, tag="sc")
                        nc.vector.tensor_add(
                            sc[:, 0 : ns * H],
                            asr[:, 0 : ns * H],
                            adx[:, 0 : ns * H],
                        )
                        lr = wpool.tile([128, SUBS * H], f32, tag="lr")
                        nc.vector.scalar_tensor_tensor(
                            out=lr[:, 0 : ns * H], in0=sc[:, 0 : ns * H],
                            scalar=NEG_SLOPE, in1=sc[:, 0 : ns * H],
                            op0=MULT, op1=MAX,
                        )
                        ex = wpool.tile([128, SUBS * H], f32, tag="ex")
                        nc.scalar.activation(
                            ex[:, 0 : ns * H], lr[:, 0 : ns * H],
                            mybir.ActivationFunctionType.Exp,
                        )
                        # msg[:, s, 0:HD] = h_src * e ; msg[:, s, HD:HD+H] = e
                        msg = wpool.tile([128, SUBS * (HD + H)], f32, tag="msg")
                        mv = msg[:].rearrange("p (s c) -> p s c", s=SUBS)
                        nc.vector.tensor_copy(
                            mv[:, 0:ns, HD : HD + H],
                            ex[:, 0 : ns * H].rearrange("p (s h) -> p s h", h=H),
                        )
                        nc.vector.tensor_mul(
                            mv[:, 0:ns, 0:HD].rearrange("p s (h d) -> p s h d", d=D),
                            gv[:, s0 : s0 + ns, 0:HD].rearrange(
                                "p s (h d) -> p s h d", d=D
                            ),
                            mv[:, 0:ns, HD : HD + H].unsqueeze(-1).to_broadcast(
                                [128, ns, H, D]
                            ),
                        )
                        # accumulate [sum(e*h) | sum(e)] over the window
                        for s in range(ns):
                            q = s0 + s
                            nc.tensor.matmul(
                                accdns[:],
                                lhsT=oh[:, s * 128 : (s + 1) * 128],
                                rhs=mv[:, s, :],
                                start=(q == 0),
                                stop=(q == kj - 1),
                            )
                    # finalize: out = acc / (dns + eps) + bias
                    dnse = fpool.tile([128, H], f32, tag="dnse")
                    nc.vector.tensor_scalar_add(dnse[:], accdns[:, HD : HD + H], EPS)
                    dnr = fpool.tile([128, H], f32, tag="dnr")
                    nc.vector.reciprocal(dnr[:], dnse[:])
                    outw = fpool.tile([128, HD], f32, tag="outw")
                    nc.vector.tensor_mul(
                        outw[:].rearrange("p (h d) -> p h d", d=D),
                        accdns[:, 0:HD].rearrange("p (h d) -> p h d", d=D),
                        dnr[:].unsqueeze(-1).to_broadcast([128, H, D]),
                    )
                    nc.vector.tensor_add(outw[:], outw[:], bias_bc[:])
                    nc.sync.dma_start(out_d[w * 128 : (w + 1) * 128, :], outw[:])
    nc.compile()
    # compile()'s late passes (act-table loads, hostgen rebases) can leave
    # >1-wait instructions behind; one more split pass clears them (the TRN2
    # ISA allows a single sem wait per compute instruction).
    nc.generate_event_semaphores()
    return nc


def kernel(x, edge_index, W, att_src, att_dst, bias):
    global LAST_RESULTS
    from concourse.bass_utils import run_bass_kernel_spmd

    x = np.asarray(x, dtype=np.float32)
    edge_index = np.asarray(edge_index)
    W = np.asarray(W, dtype=np.float32)
    att_src = np.asarray(att_src, dtype=np.float32)
    att_dst = np.asarray(att_dst, dtype=np.float32)
    bias = np.asarray(bias, dtype=np.float32)

    ilow, ihigh, dstrel, KL, KH = _prep_host(edge_index)

    xT = np.zeros((IN_DIM, NROWS_ALL), dtype=np.float32)
    xT[:, :N] = x.T
    xT = np.ascontiguousarray(xT)
    asrc_row = np.ascontiguousarray(att_src.reshape(1, HD))
    adst_row = np.ascontiguousarray(att_dst.reshape(1, HD))
    bias_row = np.ascontiguousarray(bias.reshape(1, HD))

    nc = _build_program(KL, KH)

    in_maps = []
    for c in range(NCORES):
        xdT = np.zeros((IN_DIM, WROWS), dtype=np.float32)
        xdT[:, :NPC] = x[c * NPC : (c + 1) * NPC].T
        in_maps.append(
            {
                "xT": xT,
                "xdstT": np.ascontiguousarray(xdT),
                "W": W,
                "att_src": asrc_row,
                "att_dst": adst_row,
                "bias": bias_row,
                "ilow": np.ascontiguousarray(ilow[c]),
                "ihigh": np.ascontiguousarray(ihigh[c]),
                "dstrel": np.ascontiguousarray(dstrel[c]),
            }
        )

    res = run_bass_kernel_spmd(nc, in_maps, list(range(NCORES)))
    LAST_RESULTS = res

    out = np.empty((N, HD), dtype=np.float32)
    for c in range(NCORES):
        out[c * NPC : (c + 1) * NPC] = res.results[c]["out"][:NPC]
    return out



# revision 7
# speedup vs baseline: 1.3840x; 1.3840x over previous
"""GAT layer (PyG-style, add_self_loops=True) on 8 Trainium2 NeuronCores.

Strategy: partition destination nodes (and their incident edges) across the 8
cores; each core owns a contiguous range of 6250 dst nodes (49 windows of 128).

No projection table. Per window of 128 dst nodes, two transposed dma_gathers
(lo/hi halves of the node range, int16-index limit) pull the raw 256-byte x
rows of all incident edges' sources straight out of HBM, TRANSPOSED at u16
granularity: the host pre-interleaves each x row's bytes as
[hi16(x_0)..hi16(x_63) | lo16(x_0)..lo16(x_63)], so gather partitions 0:64
hold truncated-bf16 features and serve directly as the matmul lhsT. Each
128-edge subtile then computes h|a_src = x_src @ [W | W@att_src^T] as ONE bf16
matmul (f32 PSUM), so there is no replicated 50k-row projection pass and no
38 MB table write at all.

Per-edge a_dst: one-hot(edge->dst slot) built on DVE in bf16, PE-transposed,
then a tiny matmul against the window's a_dst vector (phase-1b: 49 small
matmuls over the core's own dst shard). exp(leaky_relu(score)) on DVE+Act.
Segment-sum of [e*h | e] via PSUM-accumulated one-hot matmuls; final
out = acc/(denom+eps) + bias.  Softmax max-subtraction is skipped
(shift-invariant; scores are O(1) so exp cannot overflow).

Pad edges gather row 0 (finite) and carry dst-slot sentinel 500 => their
one-hot row is all zero, so they contribute to nothing. No dummy rows.

Host does only index/byte-space work (self-loop append, dst sort, windowing,
padding, int16 index wrapping, u16 byte interleave of x, x transpose/slice).
"""

import math

import numpy as np

N = 50000
IN_DIM = 64
H = 4
D = 64
HD = H * D  # 256
WCOLS = HD + H  # 260: per-edge matmul output h | a_src
NEG_SLOPE = 0.2
EPS = 1e-16
SENT = 500.0  # dst-slot sentinel for pad edges (one-hot row all zero)

NCORES = 8
NPC = N // NCORES  # 6250 dst nodes per core
NWIN = math.ceil(NPC / 128)  # 49 windows
WROWS = NWIN * 128  # 6272
SPLIT = 25088  # lo/hi x-table split (int16 gather index limit)
SUBS = 2  # edge subtiles per chunk (PSUM-bank budget)
CSTRIDE = 512  # psum cols per subtile slot (bank-aligned; 260 used)
ADX0 = 260  # col in each subtile's psum slot where a_dst-per-edge lands
# (contiguous with a_src at 256:260 so score = reduce_sum over the pair
#  reads PSUM with a single input AP -- DVE allows only one PSUM operand)

LAST_RESULTS = None  # BassKernelResults of the most recent run (for test.py)


def _wrap_idx(ids):
    """[n] int -> dma_gather wrapped layout [128, n/16] int16
    (idx i at [i%16, i//16], replicated across the 8 Q7 core groups)."""
    n = len(ids)
    w16 = ids.reshape(n // 16, 16).T.astype(np.int16)  # [16, n/16]
    return np.tile(w16, (8, 1))


def _interleave_x(x):
    """[N,64] f32 -> [N,128] u16 rows [hi16(x_0..63) | lo16(x_0..63)].
    After the u16-granularity transposed gather, partitions 0:64 hold the
    high halves = truncated-bf16 feature values."""
    xu = np.ascontiguousarray(x).view(np.uint16).reshape(-1, 64, 2)
    return np.ascontiguousarray(np.concatenate([xu[:, :, 1], xu[:, :, 0]], axis=1))


def _prep_host(edge_index):
    """Returns ilow  int16 [NCORES, NWIN, 128, KL*8]
               ihigh int16 [NCORES, NWIN, 128, KH*8]
               dstrel f32  [NCORES, NWIN, 128, KL+KH]  (slot or SENT)
               (KL, KH)"""
    src = np.concatenate([edge_index[0], np.arange(N, dtype=np.int64)]).astype(np.int64)
    dst = np.concatenate([edge_index[1], np.arange(N, dtype=np.int64)]).astype(np.int64)
    order = np.argsort(dst, kind="stable")
    src = src[order].astype(np.int32)
    dst = dst[order].astype(np.int32)

    bounds = [c * NPC + w * 128 for c in range(NCORES) for w in range(NWIN)]
    bounds.append(N)
    cuts = np.searchsorted(dst, np.asarray(bounds))

    lo_counts = np.zeros(NCORES * NWIN, np.int64)
    hi_counts = np.zeros(NCORES * NWIN, np.int64)
    for b in range(NCORES * NWIN):
        s = src[cuts[b] : cuts[b + 1]]
        lo_counts[b] = int((s < SPLIT).sum())
        hi_counts[b] = len(s) - lo_counts[b]
    KL = max(1, math.ceil(lo_counts.max() / 128))
    KH = max(1, math.ceil(hi_counts.max() / 128))
    kj = KL + KH

    ilow = np.zeros((NCORES, NWIN, KL * 128), np.int32)  # pad -> row 0 (finite)
    ihigh = np.zeros((NCORES, NWIN, KH * 128), np.int32)
    dstrel = np.full((NCORES, NWIN, 128, kj), SENT, np.float32)
    for c in range(NCORES):
        base = c * NPC
        for w in range(NWIN):
            b = c * NWIN + w
            s = src[cuts[b] : cuts[b + 1]]
            d = dst[cuts[b] : cuts[b + 1]] - base - w * 128
            m = s < SPLIT
            slo, dlo = s[m], d[m]
            shi, dhi = s[~m] - SPLIT, d[~m]
            # ascending source rows => HBM page locality in the gather
            o = np.argsort(slo, kind="stable")
            slo, dlo = slo[o], dlo[o]
            o = np.argsort(shi, kind="stable")
            shi, dhi = shi[o], dhi[o]
            ilow[c, w, : len(slo)] = slo
            ihigh[c, w, : len(shi)] = shi
            i = np.arange(len(slo))
            dstrel[c, w, i % 128, i // 128] = dlo
            i = np.arange(len(shi))
            dstrel[c, w, i % 128, KL + i // 128] = dhi
    ilow_w = np.zeros((NCORES, NWIN, 128, KL * 8), np.int16)
    ihigh_w = np.zeros((NCORES, NWIN, 128, KH * 8), np.int16)
    for c in range(NCORES):
        for w in range(NWIN):
            ilow_w[c, w] = _wrap_idx(ilow[c, w])
            ihigh_w[c, w] = _wrap_idx(ihigh[c, w])
    return ilow_w, ihigh_w, dstrel, KL, KH


def _build_program(KL, KH, ablate="full"):
    import concourse.bass as bass
    import concourse.bacc as bacc
    import concourse.tile as tile
    from concourse import mybir
    from concourse.masks import make_identity

    f32 = mybir.dt.float32
    bf16 = mybir.dt.bfloat16
    u16 = mybir.dt.uint16
    i16 = mybir.dt.int16
    i32 = mybir.dt.int32
    kj = KL + KH

    nc = bacc.Bacc(None, target_bir_lowering=False)

    xlo_d = nc.dram_tensor("x_lo", [SPLIT, 128], u16, kind="ExternalInput")
    xhi_d = nc.dram_tensor("x_hi", [N - SPLIT, 128], u16, kind="ExternalInput")
    xdT_d = nc.dram_tensor("xdstT", [IN_DIM, WROWS], f32, kind="ExternalInput")
    W_d = nc.dram_tensor("W", [IN_DIM, HD], f32, kind="ExternalInput")
    asrc_d = nc.dram_tensor("att_src", [1, HD], f32, kind="ExternalInput")
    adst_d = nc.dram_tensor("att_dst", [1, HD], f32, kind="ExternalInput")
    bias_d = nc.dram_tensor("bias", [1, HD], f32, kind="ExternalInput")
    il_d = nc.dram_tensor("ilow", [NWIN, 128, KL * 8], i16, kind="ExternalInput")
    ih_d = nc.dram_tensor("ihigh", [NWIN, 128, KH * 8], i16, kind="ExternalInput")
    drel_d = nc.dram_tensor("dstrel", [NWIN, 128, kj], f32, kind="ExternalInput")
    out_d = nc.dram_tensor("out", [WROWS, HD], f32, kind="ExternalOutput")

    X = mybir.AxisListType.X
    EQ = mybir.AluOpType.is_equal
    MULT = mybir.AluOpType.mult
    MAX = mybir.AluOpType.max

    with tile.TileContext(nc) as tc:
        with tc.tile_pool(name="const", bufs=1) as cpool:
            spsum_cm = tc.tile_pool(name="setup_psum", bufs=1, space="PSUM")
            spsum = spsum_cm.__enter__()
            ones = cpool.tile([1, 128], f32)
            nc.vector.memset(ones[:], 1.0)
            ident_f = cpool.tile([128, 128], f32)
            make_identity(nc, ident_f[:])
            ident = cpool.tile([128, 128], bf16)
            nc.vector.tensor_copy(ident[:], ident_f[:])
            iota_i = cpool.tile([128, 128], i32)
            nc.gpsimd.iota(iota_i[:], pattern=[[1, 128]], base=0, channel_multiplier=0)
            iota_f = cpool.tile([128, 128], bf16)
            nc.vector.tensor_copy(iota_f[:], iota_i[:])

            # WA = [W | Wsrc], Wsrc[k,h] = sum_d W[k,h*D+d]*att_src[h,d];
            # Wdst likewise (kept f32 for the per-window a_dst matmuls).
            wa_tmp = cpool.tile([IN_DIM, WCOLS], f32)
            nc.vector.memset(wa_tmp[:], 0.0)
            nc.sync.dma_start(wa_tmp[:, 0:HD], W_d[:, :])
            wdst = cpool.tile([IN_DIM, H], f32)
            att_s_raw = cpool.tile([1, HD], f32)
            nc.sync.dma_start(att_s_raw[:], asrc_d[:, :])
            att_t_raw = cpool.tile([1, HD], f32)
            nc.sync.dma_start(att_t_raw[:], adst_d[:, :])
            att_s = cpool.tile([1, HD], f32)
            nc.vector.tensor_copy(att_s[:], att_s_raw[:])
            att_t = cpool.tile([1, HD], f32)
            nc.vector.tensor_copy(att_t[:], att_t_raw[:])
            for att_tile, dst_ap in ((att_s, wa_tmp[:, HD : HD + H]), (att_t, wdst[:, :])):
                attb = spsum.tile([IN_DIM, HD], f32, tag="attb")
                nc.tensor.matmul(
                    attb[:], lhsT=ones[:1, 0:IN_DIM], rhs=att_tile[:],
                    start=True, stop=True,
                )
                tmp = cpool.tile([IN_DIM, HD], f32, tag="tmp")
                nc.vector.tensor_mul(tmp[:], wa_tmp[:, 0:HD], attb[:])
                nc.vector.reduce_sum(
                    out=dst_ap,
                    in_=tmp[:].rearrange("k (h d) -> k h d", d=D),
                    axis=X,
                )
            WAb = cpool.tile([IN_DIM, WCOLS], bf16)
            nc.vector.tensor_copy(WAb[:], wa_tmp[:])

            bias_raw = cpool.tile([1, HD], f32)
            nc.sync.dma_start(bias_raw[:], bias_d[:, :])
            bias_sb = cpool.tile([1, HD], f32)
            nc.vector.tensor_copy(bias_sb[:], bias_raw[:])
            bb = spsum.tile([128, HD], f32)
            nc.tensor.matmul(bb[:], lhsT=ones[:1, :], rhs=bias_sb[:], start=True, stop=True)
            bias_bc = cpool.tile([128, HD], f32)
            nc.scalar.copy(bias_bc[:], bb[:])

            # phase 1b: a_dst for the core's own dst shard, bf16 [128, NWIN*H]
            adst_all = cpool.tile([128, NWIN * H], bf16)

            spsum_cm.__exit__(None, None, None)  # free setup PSUM banks

            with (
                tc.tile_pool(name="p1", bufs=4) as p1,
                tc.tile_pool(name="p1ps", bufs=4, space="PSUM") as p1ps,
            ):
                for w in range(NWIN):
                    xd = p1.tile([IN_DIM, 128], f32, tag="xdr")
                    nc.sync.dma_start(xd[:], xdT_d[:, w * 128 : (w + 1) * 128])
                    adp = p1ps.tile([128, H], f32, tag="adp")
                    nc.tensor.matmul(
                        adp[:], lhsT=xd[:], rhs=wdst[:], start=True, stop=True,
                    )
                    nc.vector.tensor_copy(adst_all[:, w * H : (w + 1) * H], adp[:])

            # ---------------- main loop: per-window aggregation ----------------
            if ablate == "p1":
                with tc.tile_pool(name="fin0", bufs=1) as f0:
                    zo = f0.tile([128, HD], f32)
                    nc.vector.memset(zo[:], 0.0)
                    for w in range(NWIN):
                        nc.sync.dma_start(out_d[w * 128 : (w + 1) * 128, :], zo[:])
            nch = math.ceil(kj / SUBS)
            with (
                tc.tile_pool(name="gat", bufs=2) as gpool,
                tc.tile_pool(name="edg", bufs=2) as epool,
                tc.tile_pool(name="wrk", bufs=3) as wpool,
                tc.tile_pool(name="fin", bufs=2) as fpool,
                tc.tile_pool(name="hps", bufs=2, space="PSUM") as hpool,
                tc.tile_pool(name="sps", bufs=2, space="PSUM") as spool,
                tc.tile_pool(name="acc", bufs=2, space="PSUM") as apool,
            ):
                for w in range(NWIN if ablate != "p1" else 0):
                    il_t = epool.tile([128, KL * 8], i16, tag="il")
                    nc.sync.dma_start(il_t[:], il_d[w])
                    ih_t = epool.tile([128, KH * 8], i16, tag="ih")
                    nc.sync.dma_start(ih_t[:], ih_d[w])
                    drel_raw = epool.tile([128, kj], f32, tag="drel")
                    nc.sync.dma_start(drel_raw[:], drel_d[w])
                    drel_b = epool.tile([128, kj], bf16, tag="drelb")
                    nc.vector.tensor_copy(drel_b[:], drel_raw[:])

                    g = gpool.tile([128, kj * 128], u16, tag="g")
                    gv = g[:].unsqueeze(1)  # [128, 1, kj*128]
                    nc.gpsimd.dma_gather(
                        out_ap=gv[:, :, 0 : KL * 128], in_ap=xlo_d[:, :],
                        idxs_ap=il_t[:], num_idxs=KL * 128, num_idxs_reg=KL * 128,
                        elem_size=128, transpose=True, single_packet=False,
                    )
                    nc.gpsimd.dma_gather(
                        out_ap=gv[:, :, KL * 128 : kj * 128], in_ap=xhi_d[:, :],
                        idxs_ap=ih_t[:], num_idxs=KH * 128, num_idxs_reg=KH * 128,
                        elem_size=128, transpose=True, single_packet=False,
                    )
                    gb = g[:].bitcast(bf16)
                    adw = adst_all[:, w * H : (w + 1) * H]

                    if ablate == "p1g":
                        outw = fpool.tile([128, HD], f32, tag="outw")
                        nc.vector.memset(outw[:], 0.0)
                        nc.vector.tensor_copy(
                            outw[0:IN_DIM, 0:HD], gb[0:IN_DIM, 0:HD]
                        )
                        nc.sync.dma_start(out_d[w * 128 : (w + 1) * 128, :], outw[:])
                        continue

                    accdns = apool.tile([128, WCOLS], f32, tag="accdns")
                    for ch in range(nch):
                        s0 = ch * SUBS
                        ns = min(SUBS, kj - s0)
                        # per-edge projection: hps[:, s*512 : s*512+260] =
                        #   [h | a_src] of subtile s (bf16 matmul, f32 psum)
                        hps = hpool.tile([128, SUBS * CSTRIDE], f32, tag="hps")
                        for s in range(ns):
                            nc.tensor.matmul(
                                hps[:, s * CSTRIDE : s * CSTRIDE + WCOLS],
                                lhsT=gb[0:IN_DIM, (s0 + s) * 128 : (s0 + s + 1) * 128],
                                rhs=WAb[:],
                                start=True, stop=True,
                            )
                        # one-hot[e, s, p] = (dstrel[e, s] == p), bf16
                        oh = wpool.tile([128, SUBS * 128], bf16, tag="oh")
                        nc.vector.tensor_tensor(
                            out=oh[:, 0 : ns * 128].rearrange("p (s e) -> p s e", s=ns),
                            in0=drel_b[:, s0 : s0 + ns].unsqueeze(-1).to_broadcast(
                                [128, ns, 128]
                            ),
                            in1=iota_f[:].unsqueeze(1).to_broadcast([128, ns, 128]),
                            op=EQ,
                        )
                        # transposed one-hot (for a_dst expansion) + a_dst matmul
                        # (adx lands in unused hps columns ADX0:ADX0+H per slot
                        # -- no extra PSUM bank needed)
                        ohts = spool.tile([128, SUBS * 128], bf16, tag="ohts")
                        for s in range(ns):
                            nc.tensor.transpose(
                                ohts[:, s * 128 : (s + 1) * 128],
                                oh[:, s * 128 : (s + 1) * 128],
                                ident[:],
                            )
                        ohT = wpool.tile([128, SUBS * 128], bf16, tag="ohT")
                        nc.scalar.copy(ohT[:, 0 : ns * 128], ohts[:, 0 : ns * 128])
                        for s in range(ns):
                            nc.tensor.matmul(
                                hps[:, s * CSTRIDE + ADX0 : s * CSTRIDE + ADX0 + H],
                                lhsT=ohT[:, s * 128 : (s + 1) * 128],
                                rhs=adw,
                                start=True, stop=True,
                            )
                        # score = a_src + a_dst -> leaky_relu -> exp (bf16)
                        hv = hps[:].rearrange("p (s c) -> p s c", c=CSTRIDE)
                        sc = wpool.tile([128, SUBS * H], f32, tag="sc")
                        nc.vector.reduce_sum(
                            out=sc[:, 0 : ns * H].rearrange("p (s h) -> p s h", h=H),
                            in_=hv[:, 0:ns, HD : HD + 2 * H].rearrange(
                                "p s (a h) -> p s h a", a=2
                            ),
                            axis=X,
                        )
                        lr = wpool.tile([128, SUBS * H], f32, tag="lr")
                        nc.vector.scalar_tensor_tensor(
                            out=lr[:, 0 : ns * H], in0=sc[:, 0 : ns * H],
                            scalar=NEG_SLOPE, in1=sc[:, 0 : ns * H],
                            op0=MULT, op1=MAX,
                        )
                        ex = wpool.tile([128, SUBS * H], bf16, tag="ex")
                        nc.scalar.activation(
                            ex[:, 0 : ns * H], lr[:, 0 : ns * H],
                            mybir.ActivationFunctionType.Exp,
                        )
                        # msg[:, s, 0:HD] = h_s * e ; msg[:, s, HD:HD+H] = e
                        msg = wpool.tile([128, SUBS * WCOLS], bf16, tag="msg")
                        mv = msg[:].rearrange("p (s c) -> p s c", s=SUBS)
                        nc.vector.tensor_copy(
                            mv[:, 0:ns, HD : HD + H],
                            ex[:, 0 : ns * H].rearrange("p (s h) -> p s h", h=H),
                        )
                        nc.vector.tensor_mul(
                            mv[:, 0:ns, 0:HD].rearrange("p s (h d) -> p s h d", d=D),
                            hps[:].rearrange("p (s c) -> p s c", c=CSTRIDE)[
                                :, 0:ns, 0:HD
                            ].rearrange("p s (h d) -> p s h d", d=D),
                            mv[:, 0:ns, HD : HD + H].unsqueeze(-1).to_broadcast(
                                [128, ns, H, D]
                            ),
                        )
                        # accumulate [sum(e*h) | sum(e)] over the window
                        for s in range(ns):
                            q = s0 + s
                            nc.tensor.matmul(
                                accdns[:],
                                lhsT=oh[:, s * 128 : (s + 1) * 128],
                                rhs=mv[:, s, :],
                                start=(q == 0),
                                stop=(q == kj - 1),
                            )
                    # finalize: out = acc / (dns + eps) + bias
                    dnse = fpool.tile([128, H], f32, tag="dnse")
                    nc.vector.tensor_scalar_add(dnse[:], accdns[:, HD : HD + H], EPS)
                    dnr = fpool.tile([128, H], f32, tag="dnr")
                    nc.vector.reciprocal(dnr[:], dnse[:])
                    outw = fpool.tile([128, HD], f32, tag="outw")
                    nc.vector.tensor_mul(
                        outw[:].rearrange("p (h d) -> p h d", d=D),
                        accdns[:, 0:HD].rearrange("p (h d) -> p h d", d=D),
                        dnr[:].unsqueeze(-1).to_broadcast([128, H, D]),
                    )
                    nc.vector.tensor_add(outw[:], outw[:], bias_bc[:])
                    nc.sync.dma_start(out_d[w * 128 : (w + 1) * 128, :], outw[:])
    nc.compile()
    # compile()'s late passes (act-table loads, hostgen rebases) can leave
    # >1-wait instructions behind; one more split pass clears them (the TRN2
    # ISA allows a single sem wait per compute instruction).
    nc.generate_event_semaphores()
    return nc


def _stage_inputs(x, W, att_src, att_dst, bias, ilow, ihigh, dstrel):
    x = np.asarray(x, dtype=np.float32)
    x_il = _interleave_x(x)
    x_lo = np.ascontiguousarray(x_il[:SPLIT])
    x_hi = np.ascontiguousarray(x_il[SPLIT:])
    asrc_row = np.ascontiguousarray(np.asarray(att_src, np.float32).reshape(1, HD))
    adst_row = np.ascontiguousarray(np.asarray(att_dst, np.float32).reshape(1, HD))
    bias_row = np.ascontiguousarray(np.asarray(bias, np.float32).reshape(1, HD))
    in_maps = []
    for c in range(NCORES):
        xdT = np.zeros((IN_DIM, WROWS), dtype=np.float32)
        xdT[:, :NPC] = x[c * NPC : (c + 1) * NPC].T
        in_maps.append(
            {
                "x_lo": x_lo,
                "x_hi": x_hi,
                "xdstT": np.ascontiguousarray(xdT),
                "W": np.asarray(W, np.float32),
                "att_src": asrc_row,
                "att_dst": adst_row,
                "bias": bias_row,
                "ilow": np.ascontiguousarray(ilow[c]),
                "ihigh": np.ascontiguousarray(ihigh[c]),
                "dstrel": np.ascontiguousarray(dstrel[c]),
            }
        )
    return in_maps


def kernel(x, edge_index, W, att_src, att_dst, bias):
    global LAST_RESULTS
    from concourse.bass_utils import run_bass_kernel_spmd

    edge_index = np.asarray(edge_index)
    ilow, ihigh, dstrel, KL, KH = _prep_host(edge_index)
    nc = _build_program(KL, KH)
    in_maps = _stage_inputs(x, W, att_src, att_dst, bias, ilow, ihigh, dstrel)

    res = run_bass_kernel_spmd(nc, in_maps, list(range(NCORES)))
    LAST_RESULTS = res

    out = np.empty((N, HD), dtype=np.float32)
    for c in range(NCORES):
        out[c * NPC : (c + 1) * NPC] = res.results[c]["out"][:NPC]
    return out
